# revision 33
# baseline (speedup 1.0000x reference)
"""Multi-head attention (B=4, S=2048, D=1024, H=16) on 8 trn2 NeuronCores.

Sharding: core c = (batch b, head-group g) with b in 0..3, g in 0..1.
Each core computes 8 heads of one batch; the two cores of a batch produce
partial output projections that the host sums.

All device tensors are kept in "transposed" layouts (feature dim on SBUF
partitions) so no on-device transposes are needed:
  Q^T/K^T [d, s], V [s, d] (+ones col), scores^T [k, q], o^T [d, q], y^T [out, q].
Softmax uses no max-subtraction (scores bounded ~ +-10 for this regime) and
the denominator comes from an appended ones-column in the PV matmul.

Schedule: pair-outer iteration (iter = 4*pair + qc).  The two K=64 score
matmuls of a head pair run CONCURRENTLY on disjoint PE row-halves (bass
auto-infers tile_position from base partitions).  All projection work
(K/Q/V/Wo) drains into the attention stream as SINGLE-matmul items paced
by per-item deadlines, so the activation engine (exp, the second-longest
pole) is never starved for long.  PV emission lags the score stream by
PV_LAG kt-slots so the softmax-normalization chain of the previous
iteration (reciprocal -> PE-broadcast -> o-multiply) finishes before its
PSUM accumulator is reused.
"""
import math

import numpy as np
import ml_dtypes

import concourse.bass as bass
import concourse.mybir as mybir
import concourse.tile as tile
from concourse import bacc
from concourse.bass_utils import run_bass_kernel_spmd

B, S, D, H = 4, 2048, 1024, 16
DK = D // H              # 64
NCORES = 8
HG = 2                   # head groups (tensor-parallel axis)
HPG = H // HG            # 8 heads per core
HD = HPG * DK            # 512 head-dim features per core
PAIRS = HPG // 2         # 4 head pairs (2 heads row-packed per PE pass)
P = 128
QC = 512                 # q-chunk (matmul moving free dim)
NQC = S // QC            # 4
NKT = S // P             # 16 k-tiles
FK = D // P              # 8 feature c-tiles for projections
TC = 512                 # token chunk for QKV phase
NTC = S // TC            # 4
PV_LAG = 7               # kt-slots of lag between score and PV streams

F32 = mybir.dt.float32
BF16 = mybir.dt.bfloat16

LAST_EXEC_NS = None


def _build(apply_mask: bool, qkv_bias: bool):
    nc = bacc.Bacc("TRN2", debug=False, num_devices=NCORES)
    xT = nc.declare_dram_parameter("xT", [D, S], BF16, isOutput=False)
    wqkv = nc.declare_dram_parameter("wqkv", [D, 3 * HD], BF16, isOutput=False)
    wo = nc.declare_dram_parameter("wo", [HD, D], BF16, isOutput=False)
    yT = nc.declare_dram_parameter("yT", [D, S], BF16, isOutput=True)
    if apply_mask:
        maskT = nc.declare_dram_parameter("maskT", [S, S], BF16, isOutput=False)
    if qkv_bias:
        qkb = nc.declare_dram_parameter("qkb", [2, HD], F32, isOutput=False)
        vb = nc.declare_dram_parameter("vb", [HD], F32, isOutput=False)

    xT_r = xT.rearrange("(fo p) s -> p fo s", p=P)       # [128, 8, 2048]
    wqkv_r = wqkv.rearrange("(fo p) n -> p fo n", p=P)   # [128, 8, 1536]
    wo_r = wo.rearrange("(co p) n -> p co n", p=P)       # [128, 4, 1024]
    yT_r = yT.rearrange("(oo p) s -> p oo s", p=P)       # [128, 8, 2048]

    phat_bufs = 1 if apply_mask else 2

    with tile.TileContext(nc) as tc:
        with tc.tile_pool(name="persist", bufs=1) as persist, \
             tc.tile_pool(name="work", bufs=2) as work, \
             tc.tile_pool(name="small", bufs=1) as small, \
             tc.tile_pool(name="phat", bufs=phat_bufs) as phatp, \
             tc.tile_pool(name="opool", bufs=2) as opool, \
             tc.tile_pool(name="ps", bufs=2, space="PSUM") as ps:

            QT = persist.tile([P, PAIRS, S], BF16)        # 16KB/part
            KTt = persist.tile([P, PAIRS, S], BF16)       # 16KB/part
            V = persist.tile([P, NKT, HPG, DK + 1], BF16)  # 16.25KB/part
            wo_t = persist.tile([P, HD // P, D], BF16)    # 8KB/part
            o_t = [persist.tile([P, HD // P, QC], BF16, name=f"o_t{qc}")
                   for qc in range(NQC)]                  # 16KB/part total

            if qkv_bias:
                qkb_t = persist.tile([P, 2, PAIRS], F32)
                nc.sync.dma_start(
                    qkb_t, qkb.rearrange("t (pr p) -> p t pr", p=P))
                vb_bc = persist.tile([P, HD], F32)
                nc.sync.dma_start(vb_bc, vb[None, :].partition_broadcast(P))

            # ones columns of V (softmax-denominator trick)
            nc.vector.memset(V[:, :, :, DK], 1.0)

            # x and weights stay resident as per-ko tiles.  First halves of
            # x issue before second halves; the K third of wqkv first.
            x_ko, w_ko = [], []
            for ko in range(FK):
                xk = persist.tile([P, S], BF16, name=f"x_ko{ko}")   # 4KB each
                x_ko.append(xk)
                wk = persist.tile([P, 3 * HD], BF16, name=f"w_ko{ko}")  # 3KB
                w_ko.append(wk)
            for ko in range(FK):   # K-projection weights first
                nc.sync.dma_start(w_ko[ko][:, HD:2 * HD],
                                  wqkv_r[:, ko, HD:2 * HD])
            for c in range(4):     # x in column chunks, ko-major
                csl = slice(c * TC, (c + 1) * TC)
                for ko in range(FK):
                    nc.sync.dma_start(x_ko[ko][:, csl], xT_r[:, ko, csl])
                if c == 0:
                    for ko in range(FK):
                        nc.sync.dma_start(w_ko[ko][:, 0:HD],
                                          wqkv_r[:, ko, 0:HD])
            for ko in range(FK):
                nc.sync.dma_start(w_ko[ko][:, 2 * HD:3 * HD],
                                  wqkv_r[:, ko, 2 * HD:3 * HD])
            nc.sync.dma_start(wo_t, wo_r)

            written = set()
            tails_done = {qc: 0 for qc in range(NQC)}

            # ---- single-matmul work items --------------------------------
            def qk_singles(which, pair, tcix):
                tsl = slice(tcix * TC, (tcix + 1) * TC)
                base = 0 if which == 0 else HD
                msl = slice(base + pair * P, base + (pair + 1) * P)
                st = {}

                def mk(ko):
                    def f():
                        if ko == 0:
                            st["ps"] = ps.tile([P, TC], F32, tag="proj",
                                               name="psqk")
                        nc.tensor.matmul(
                            st["ps"], w_ko[ko][:, msl], x_ko[ko][:, tsl],
                            start=(ko == 0), stop=(ko == FK - 1))
                        if ko == FK - 1:
                            dst = (QT if which == 0 else KTt)[:, pair, tsl]
                            if qkv_bias:
                                nc.vector.tensor_scalar_add(
                                    dst, st["ps"], qkb_t[:, which, pair, None])
                            else:
                                nc.vector.tensor_copy(dst, st["ps"])
                            written.add((("q", "k")[which], pair, tcix))
                    return f
                return [mk(ko) for ko in range(FK)]

            def v_singles(kt):
                st = {}

                def mk(ko):
                    def f():
                        if ko == 0:
                            st["ps"] = ps.tile([P, HD], F32, tag="proj",
                                               name="psv")
                        nc.tensor.matmul(
                            st["ps"], x_ko[ko][:, kt * P:(kt + 1) * P],
                            w_ko[ko][:, 2 * HD:3 * HD],
                            start=(ko == 0), stop=(ko == FK - 1))
                        if ko == FK - 1:
                            vdst = V[:, kt, :, :DK]
                            vsrc = st["ps"].rearrange("p (h w) -> p h w",
                                                      h=HPG)
                            if qkv_bias:
                                nc.vector.tensor_add(
                                    vdst, vsrc,
                                    vb_bc.rearrange("p (h w) -> p h w", h=HPG))
                            else:
                                nc.vector.tensor_copy(vdst, vsrc)
                            written.add(("v", kt))
                    return f
                return [mk(ko) for ko in range(FK)]

            def oproj_singles(qc, oc):
                qsl = slice(qc * QC, (qc + 1) * QC)
                st = {}

                def mk(c):
                    def f():
                        if c == 0:
                            assert tails_done[qc] == PAIRS, (qc, tails_done)
                            st["ps"] = ps.tile([P, QC], F32, tag="proj",
                                               name="psy")
                        nc.tensor.matmul(
                            st["ps"], wo_t[:, c, oc * P:(oc + 1) * P],
                            o_t[qc][:, c, :],
                            start=(c == 0), stop=(c == HD // P - 1))
                        if c == HD // P - 1:
                            yst = work.tile([P, QC], BF16, tag="y")
                            nc.vector.tensor_copy(yst, st["ps"])
                            nc.sync.dma_start(yT_r[:, oc, qsl], yst)
                    return f
                return [mk(c) for c in range(HD // P)]

            # ---- prologue: K^T(chunk 0) + Q^T(chunk 0) for pair 0 only --
            for f in qk_singles(1, 0, 0):
                f()
            for f in qk_singles(0, 0, 0):
                f()

            # pending: (ready_slot, deadline_slot, fn), sorted by deadline.
            # Deadlines are the global kt-slot by which the item must have
            # been EMITTED (the tile framework orders by emission, so a
            # late item means its consumer reads stale data).
            pending = []

            def it_slot(pair, qc, kt=0):
                return 16 * (4 * pair + qc) + kt

            for tcix in range(1, NTC):          # K(p0, t>=1): scores kt=4t
                for f in qk_singles(1, 0, tcix):
                    pending.append((0, it_slot(0, 0, 4 * tcix) - 2, f))
            for tcix in range(1, NTC):          # Q(p0, t>=1)
                for f in qk_singles(0, 0, tcix):
                    pending.append((0, it_slot(0, tcix) - 2, f))
            for kt in range(NKT):               # V[kt]: first PV use, iter 1
                for f in v_singles(kt):
                    pending.append(
                        (0, it_slot(0, 1, min(kt + PV_LAG, NKT - 1)) - 1, f))
            for pair in range(1, PAIRS):
                for tcix in range(NTC):         # K(p, t): scores at kt=4t
                    for f in qk_singles(1, pair, tcix):
                        pending.append((0, it_slot(pair, 0, 4 * tcix) - 2, f))
                for tcix in range(NTC):         # Q(p, t)
                    for f in qk_singles(0, pair, tcix):
                        pending.append((0, it_slot(pair, tcix) - 2, f))
            pending.sort(key=lambda x: x[1])

            cur = [0]

            def drain_auto(cap=8, base=1):
                # rate needed so every deadline in the near prefix is met
                need = 0
                for i, (rs, dl, _) in enumerate(pending):
                    if dl > cur[0] + 64:
                        break
                    slack = max(1, dl - cur[0])
                    need = max(need, -(-(i + 1) // slack))
                n = 0
                target = min(cap, max(base, need))
                while pending and n < target:
                    rs, dl, fn = pending[0]
                    if rs > cur[0]:
                        break
                    assert dl >= cur[0], f"missed deadline {dl} at {cur[0]}"
                    pending.pop(0)
                    fn()
                    n += 1

            # PV emission schedule: kt-slot -> list of lagged kt values
            pv_sched = [[] for _ in range(NKT)]
            for ktv in range(NKT):
                pv_sched[min(ktv + PV_LAG, NKT - 1)].append(ktv)

            def emit_pv(st, pso_h, half, kt):
                assert ("v", kt) in written
                hh = 2 * st["pair"] + half
                nc.tensor.matmul(
                    pso_h[0:DK + 1, :],
                    V[:, kt, hh, :],
                    st["phat"][:, kt, half * QC:(half + 1) * QC],
                    start=(kt == 0), stop=(kt == NKT - 1))

            def emit_tails(st):
                pso_h = st["pso"]
                for half in range(2):
                    l0 = small.tile([1, QC], F32, tag="l0", name="l0")
                    nc.vector.tensor_copy(l0, pso_h[half][DK:DK + 1, :])
                    r_sb = small.tile([1, QC], F32, tag="r", name="r_sb")
                    nc.vector.reciprocal_approx_fast(r_sb, l0)
                    r_bc = small.tile([DK, QC], F32, tag="rbc", name="r_bc")
                    nc.gpsimd.partition_broadcast(r_bc, r_sb)
                    nc.vector.tensor_mul(
                        o_t[st["qc"]][half * DK:(half + 1) * DK,
                                      st["pair"], :],
                        pso_h[half][0:DK, :], r_bc)
                tails_done[st["qc"]] += 1
                if st["pair"] == PAIRS - 1:
                    qc = st["qc"]
                    for oc in range(D // P):
                        for f in oproj_singles(qc, oc):
                            pending.append((cur[0] + 1, 1 << 30, f))

            # ---- main attention pipeline --------------------------------
            prev = None
            fin = {"prev_done": 0, "self_done": 0, "pso": None}
            for pair in range(PAIRS):
                for qc in range(NQC):
                    last = (pair == PAIRS - 1 and qc == NQC - 1)
                    qsl = slice(qc * QC, (qc + 1) * QC)
                    if apply_mask:
                        mt = opool.tile([P, NKT, QC], BF16, tag="mask")
                        nc.sync.dma_start(
                            mt, maskT.rearrange(
                                "(ko p) q -> p ko q", p=P)[:, :, qsl])
                    phat = phatp.tile([P, NKT, 2 * QC], BF16, tag="ph",
                                      name="phat")
                    self_st = {"qc": qc, "pair": pair, "phat": phat}
                    pso_h = None
                    for kt in range(NKT):
                        assert ("k", pair, kt * P // TC) in written
                        assert ("q", pair, qc) in written
                        ksl = slice(kt * P, (kt + 1) * P)
                        pss = ps.tile([P, 2 * QC], F32, tag="scores")
                        nc.tensor.matmul(
                            pss[:, 0:QC], KTt[0:DK, pair, ksl],
                            QT[0:DK, pair, qsl], start=True, stop=True)
                        nc.tensor.matmul(
                            pss[:, QC:2 * QC], KTt[DK:P, pair, ksl],
                            QT[DK:P, pair, qsl], start=True, stop=True)
                        if apply_mask:
                            nc.vector.tensor_add(
                                pss[:, 0:QC], pss[:, 0:QC], mt[:, kt])
                            nc.vector.tensor_add(
                                pss[:, QC:2 * QC], pss[:, QC:2 * QC],
                                mt[:, kt])
                        nc.scalar.activation(
                            phat[:, kt, :], pss,
                            mybir.ActivationFunctionType.Exp)
                        if prev is not None and not last:
                            for ktv in pv_sched[kt]:
                                if ktv == 0:
                                    pso_h = [
                                        ps.tile([P, QC], F32, tag="pso",
                                                name=f"pso{h}")
                                        for h in range(2)]
                                emit_pv(prev, pso_h[0], 0, ktv)
                                emit_pv(prev, pso_h[1], 1, ktv)
                            if kt == NKT - 1:
                                prev["pso"] = pso_h
                                emit_tails(prev)
                        elif last:
                            # burst prev's PV early, then run this iter's own
                            # PV inline as soon as each exp lands
                            if kt < 6:
                                for _ in range(3):
                                    if fin["prev_done"] < NKT:
                                        ktv = fin["prev_done"]
                                        if ktv == 0:
                                            pso_h = [
                                                ps.tile([P, QC], F32,
                                                        tag="pso",
                                                        name=f"pso{h}")
                                                for h in range(2)]
                                        emit_pv(prev, pso_h[0], 0, ktv)
                                        emit_pv(prev, pso_h[1], 1, ktv)
                                        fin["prev_done"] += 1
                            elif kt == 6:
                                assert fin["prev_done"] == NKT
                                prev["pso"] = pso_h
                                emit_tails(prev)
                            elif kt >= 9:
                                for _ in range(3):
                                    if fin["self_done"] < min(kt - 1, NKT):
                                        ktv = fin["self_done"]
                                        if ktv == 0:
                                            fin["pso"] = [
                                                ps.tile([P, QC], F32,
                                                        tag="pso",
                                                        name=f"psof{h}")
                                                for h in range(2)]
                                        emit_pv(self_st, fin["pso"][0],
                                                0, ktv)
                                        emit_pv(self_st, fin["pso"][1],
                                                1, ktv)
                                        fin["self_done"] += 1
                        drain_auto()
                        cur[0] += 1
                    prev = {"qc": qc, "pair": pair, "phat": phat}

            # ---- epilogue: finish the final iteration's PV + output ----
            for ktv in range(fin["self_done"], NKT):
                emit_pv(prev, fin["pso"][0], 0, ktv)
                emit_pv(prev, fin["pso"][1], 1, ktv)
            prev["pso"] = fin["pso"]
            emit_tails(prev)
            while pending:
                rs, dl, fn = pending.pop(0)
                fn()

    nc.finalize()
    return nc


# --------------------------------------------------------------------------
# NTFF profiling shim (only used when kernel(..., _trace=True); provides
# antenv.axon_hooks so run_bass_kernel_spmd can capture profiles under axon).
def _install_ntff_shim():
    import contextlib, ctypes, sys, types
    try:
        import antenv.axon_hooks  # noqa: F401
        return
    except ImportError:
        pass
    so = "/opt/axon/libaxon_pjrt.so"
    try:
        lib = ctypes.CDLL(so)
    except OSError:
        return
    if not hasattr(lib, "axon_start_nrt_profile"):
        return
    lib.axon_start_nrt_profile.argtypes = [
        ctypes.POINTER(ctypes.c_int64), ctypes.c_size_t]
    lib.axon_start_nrt_profile.restype = ctypes.c_int64
    lib.axon_stop_nrt_profile.argtypes = [ctypes.c_char_p]
    lib.axon_stop_nrt_profile.restype = ctypes.c_int64

    @contextlib.contextmanager
    def _hook(output_dir, device_ids):
        import jax
        jax.devices()
        if device_ids:
            ids = (ctypes.c_int64 * len(device_ids))(*device_ids)
            rc = lib.axon_start_nrt_profile(ids, len(device_ids))
        else:
            rc = lib.axon_start_nrt_profile(None, 0)
        if rc != 0:
            raise RuntimeError(f"axon_start_nrt_profile rc={rc}")
        try:
            yield
        finally:
            n = lib.axon_stop_nrt_profile(str(output_dir).encode())
            print(f"ntff: {n} profile file(s) in {output_dir}", file=sys.stderr)

    import antenv
    mod = types.ModuleType("antenv.axon_hooks")
    mod.get_axon_ntff_profile_hook = lambda: _hook
    mod.set_axon_ntff_profile_hook = lambda h: None
    sys.modules["antenv.axon_hooks"] = mod
    antenv.axon_hooks = mod


def kernel(x, mask, Wq, bq, Wk, bk, Wv, bv, Wo, bo, _trace=False):
    global LAST_EXEC_NS
    x = np.ascontiguousarray(np.asarray(x, dtype=np.float32))
    mask = np.asarray(mask)
    Wq = np.asarray(Wq, dtype=np.float32)
    Wk = np.asarray(Wk, dtype=np.float32)
    Wv = np.asarray(Wv, dtype=np.float32)
    Wo = np.asarray(Wo, dtype=np.float32)
    bq = np.asarray(bq, dtype=np.float32)
    bk = np.asarray(bk, dtype=np.float32)
    bv = np.asarray(bv, dtype=np.float32)
    bo = np.asarray(bo, dtype=np.float32)

    scale = np.float32(1.0 / math.sqrt(DK))
    apply_mask = not bool((mask != 0).all())
    qkv_bias = bool(bq.any() or bk.any() or bv.any())

    nc = _build(apply_mask, qkv_bias)

    if apply_mask:
        mbias = np.where(mask == 0, np.float32(-30000.0), np.float32(0.0))
        # maskT[b][k, q] = mbias[b][q, k]
        maskT = np.ascontiguousarray(
            np.transpose(mbias, (0, 2, 1))).astype(ml_dtypes.bfloat16)

    in_maps = []
    for b in range(B):
        xT_np = np.ascontiguousarray(x[b].T).astype(ml_dtypes.bfloat16)  # [D, S]
        for g in range(HG):
            rows = slice(g * HD, (g + 1) * HD)
            wqkv_np = np.ascontiguousarray(np.concatenate(
                [Wq[rows].T * scale, Wk[rows].T, Wv[rows].T],
                axis=1)).astype(ml_dtypes.bfloat16)
            wo_np = np.ascontiguousarray(
                Wo[:, rows].T).astype(ml_dtypes.bfloat16)
            m = {"xT": xT_np, "wqkv": wqkv_np, "wo": wo_np}
            if apply_mask:
                m["maskT"] = maskT[b]
            if qkv_bias:
                m["qkb"] = np.ascontiguousarray(
                    np.stack([bq[rows] * scale, bk[rows]]))
                m["vb"] = np.ascontiguousarray(bv[rows])
            in_maps.append(m)

    if _trace:
        _install_ntff_shim()
    r = run_bass_kernel_spmd(nc, in_maps, list(range(NCORES)), trace=_trace)
    LAST_EXEC_NS = r.exec_time_ns

    y = np.empty((B, S, D), dtype=np.float32)
    for b in range(B):
        yT = (r.results[2 * b]["yT"].astype(np.float32)
              + r.results[2 * b + 1]["yT"].astype(np.float32))
        y[b] = yT.T + bo[None, :]
    return y


# revision 34
# speedup vs baseline: 1.0273x; 1.0273x over previous
"""Multi-head attention (B=4, S=2048, D=1024, H=16) on 8 trn2 NeuronCores.

Sharding: core c = (batch b, head-group g) with b in 0..3, g in 0..1.
Each core computes 8 heads of one batch; the two cores of a batch produce
partial output projections that the host sums.

All device tensors are kept in "transposed" layouts (feature dim on SBUF
partitions) so no on-device transposes are needed:
  Q^T/K^T [d, s], V [s, d] (+ones col), scores^T [k, q], o^T [d, q], y^T [out, q].
Softmax uses no max-subtraction (scores bounded ~ +-10 for this regime) and
the denominator comes from an appended ones-column in the PV matmul.

Schedule: pair-outer iteration (iter = 4*pair + qc).  The two K=64 score
matmuls of a head pair run CONCURRENTLY on disjoint PE row-halves (bass
auto-infers tile_position from base partitions).  All projection work
(K/Q/V/Wo) drains into the attention stream as SINGLE-matmul items paced
by per-item deadlines, so the activation engine (exp, the second-longest
pole) is never starved for long.  PV emission lags the score stream by
PV_LAG kt-slots so the softmax-normalization chain of the previous
iteration (reciprocal -> PE-broadcast -> o-multiply) finishes before its
PSUM accumulator is reused.
"""
import math

import numpy as np
import ml_dtypes

import concourse.bass as bass
import concourse.mybir as mybir
import concourse.tile as tile
from concourse import bacc
from concourse.bass_utils import run_bass_kernel_spmd

B, S, D, H = 4, 2048, 1024, 16
DK = D // H              # 64
NCORES = 8
HG = 2                   # head groups (tensor-parallel axis)
HPG = H // HG            # 8 heads per core
HD = HPG * DK            # 512 head-dim features per core
PAIRS = HPG // 2         # 4 head pairs (2 heads row-packed per PE pass)
P = 128
QC = 512                 # q-chunk (matmul moving free dim)
NQC = S // QC            # 4
NKT = S // P             # 16 k-tiles
FK = D // P              # 8 feature c-tiles for projections
TC = 512                 # token chunk for QKV phase
NTC = S // TC            # 4
PV_LAG = 7               # kt-slots of lag between score and PV streams

F32 = mybir.dt.float32
BF16 = mybir.dt.bfloat16

LAST_EXEC_NS = None


def _build(apply_mask: bool, qkv_bias: bool):
    nc = bacc.Bacc("TRN2", debug=False, num_devices=NCORES)
    xT = nc.declare_dram_parameter("xT", [D, S], BF16, isOutput=False)
    wqkv = nc.declare_dram_parameter("wqkv", [D, 3 * HD], BF16, isOutput=False)
    wo = nc.declare_dram_parameter("wo", [HD, D], BF16, isOutput=False)
    yT = nc.declare_dram_parameter("yT", [D, S], BF16, isOutput=True)
    if apply_mask:
        maskT = nc.declare_dram_parameter("maskT", [S, S], BF16, isOutput=False)
    if qkv_bias:
        qkb = nc.declare_dram_parameter("qkb", [2, HD], F32, isOutput=False)
        vb = nc.declare_dram_parameter("vb", [HD], F32, isOutput=False)

    xT_r = xT.rearrange("(fo p) s -> p fo s", p=P)       # [128, 8, 2048]
    wqkv_r = wqkv.rearrange("(fo p) n -> p fo n", p=P)   # [128, 8, 1536]
    wo_r = wo.rearrange("(co p) n -> p co n", p=P)       # [128, 4, 1024]
    yT_r = yT.rearrange("(oo p) s -> p oo s", p=P)       # [128, 8, 2048]

    phat_bufs = 1 if apply_mask else 2

    with tile.TileContext(nc) as tc:
        with tc.tile_pool(name="persist", bufs=1) as persist, \
             tc.tile_pool(name="work", bufs=2) as work, \
             tc.tile_pool(name="small", bufs=1) as small, \
             tc.tile_pool(name="phat", bufs=phat_bufs) as phatp, \
             tc.tile_pool(name="opool", bufs=2) as opool, \
             tc.tile_pool(name="ps", bufs=2, space="PSUM") as ps:

            QT = persist.tile([P, PAIRS, S], BF16)        # 16KB/part
            KTt = persist.tile([P, PAIRS, S], BF16)       # 16KB/part
            V = persist.tile([P, NKT, HPG, DK + 1], BF16)  # 16.25KB/part
            wo_t = persist.tile([P, HD // P, D], BF16)    # 8KB/part
            o_t = [persist.tile([P, HD // P, QC], BF16, name=f"o_t{qc}")
                   for qc in range(NQC)]                  # 16KB/part total

            if qkv_bias:
                qkb_t = persist.tile([P, 2, PAIRS], F32)
                nc.sync.dma_start(
                    qkb_t, qkb.rearrange("t (pr p) -> p t pr", p=P))
                vb_bc = persist.tile([P, HD], F32)
                nc.sync.dma_start(vb_bc, vb[None, :].partition_broadcast(P))

            # ones columns of V (softmax-denominator trick)
            nc.vector.memset(V[:, :, :, DK], 1.0)

            # x and weights stay resident as per-ko tiles.  First halves of
            # x issue before second halves; the K third of wqkv first.
            x_ko, w_ko = [], []
            for ko in range(FK):
                xk = persist.tile([P, S], BF16, name=f"x_ko{ko}")   # 4KB each
                x_ko.append(xk)
                wk = persist.tile([P, 3 * HD], BF16, name=f"w_ko{ko}")  # 3KB
                w_ko.append(wk)
            for ko in range(FK):   # K-projection weights first
                nc.sync.dma_start(w_ko[ko][:, HD:2 * HD],
                                  wqkv_r[:, ko, HD:2 * HD])
            for c in range(4):     # x in column chunks, ko-major
                csl = slice(c * TC, (c + 1) * TC)
                for ko in range(FK):
                    nc.sync.dma_start(x_ko[ko][:, csl], xT_r[:, ko, csl])
                if c == 0:
                    for ko in range(FK):
                        nc.sync.dma_start(w_ko[ko][:, 0:HD],
                                          wqkv_r[:, ko, 0:HD])
            for ko in range(FK):
                nc.sync.dma_start(w_ko[ko][:, 2 * HD:3 * HD],
                                  wqkv_r[:, ko, 2 * HD:3 * HD])
            nc.sync.dma_start(wo_t, wo_r)

            written = set()
            tails_done = {qc: 0 for qc in range(NQC)}

            # ---- single-matmul work items --------------------------------
            def qk_singles(which, pair, tcix):
                tsl = slice(tcix * TC, (tcix + 1) * TC)
                base = 0 if which == 0 else HD
                msl = slice(base + pair * P, base + (pair + 1) * P)
                st = {}

                def mk(ko):
                    def f():
                        if ko == 0:
                            st["ps"] = ps.tile([P, TC], F32, tag="proj",
                                               name="psqk")
                        nc.tensor.matmul(
                            st["ps"], w_ko[ko][:, msl], x_ko[ko][:, tsl],
                            start=(ko == 0), stop=(ko == FK - 1))
                        if ko == FK - 1:
                            dst = (QT if which == 0 else KTt)[:, pair, tsl]
                            if qkv_bias:
                                nc.vector.tensor_scalar_add(
                                    dst, st["ps"], qkb_t[:, which, pair, None])
                            else:
                                nc.vector.tensor_copy(dst, st["ps"])
                            written.add((("q", "k")[which], pair, tcix))
                    return f
                return [mk(ko) for ko in range(FK)]

            def v_singles(kt):
                st = {}

                def mk(ko):
                    def f():
                        if ko == 0:
                            st["ps"] = ps.tile([P, HD], F32, tag="proj",
                                               name="psv")
                        nc.tensor.matmul(
                            st["ps"], x_ko[ko][:, kt * P:(kt + 1) * P],
                            w_ko[ko][:, 2 * HD:3 * HD],
                            start=(ko == 0), stop=(ko == FK - 1))
                        if ko == FK - 1:
                            vdst = V[:, kt, :, :DK]
                            vsrc = st["ps"].rearrange("p (h w) -> p h w",
                                                      h=HPG)
                            if qkv_bias:
                                nc.vector.tensor_add(
                                    vdst, vsrc,
                                    vb_bc.rearrange("p (h w) -> p h w", h=HPG))
                            else:
                                nc.vector.tensor_copy(vdst, vsrc)
                            written.add(("v", kt))
                    return f
                return [mk(ko) for ko in range(FK)]

            def oproj_singles(qc, oc):
                qsl = slice(qc * QC, (qc + 1) * QC)
                st = {}

                def mk(c):
                    def f():
                        if c == 0:
                            assert tails_done[qc] == PAIRS, (qc, tails_done)
                            st["ps"] = ps.tile([P, QC], F32, tag="proj",
                                               name="psy")
                        nc.tensor.matmul(
                            st["ps"], wo_t[:, c, oc * P:(oc + 1) * P],
                            o_t[qc][:, c, :],
                            start=(c == 0), stop=(c == HD // P - 1))
                        if c == HD // P - 1:
                            yst = work.tile([P, QC], BF16, tag="y")
                            nc.vector.tensor_copy(yst, st["ps"])
                            nc.sync.dma_start(yT_r[:, oc, qsl], yst)
                    return f
                return [mk(c) for c in range(HD // P)]

            # ---- prologue: K^T(chunk 0) + Q^T(chunk 0) for pair 0 only --
            for f in qk_singles(1, 0, 0):
                f()
            for f in qk_singles(0, 0, 0):
                f()

            # pending: (ready_slot, deadline_slot, fn), sorted by deadline.
            # Deadlines are the global kt-slot by which the item must have
            # been EMITTED (the tile framework orders by emission, so a
            # late item means its consumer reads stale data).
            pending = []

            def it_slot(pair, qc, kt=0):
                return 16 * (4 * pair + qc) + kt

            for tcix in range(1, NTC):          # K(p0, t>=1): scores kt=4t
                for f in qk_singles(1, 0, tcix):
                    pending.append((0, it_slot(0, 0, 4 * tcix) - 2, f))
            for tcix in range(1, NTC):          # Q(p0, t>=1)
                for f in qk_singles(0, 0, tcix):
                    pending.append((0, it_slot(0, tcix) - 2, f))
            for kt in range(NKT):               # V[kt]: first PV use, iter 1
                for f in v_singles(kt):
                    pending.append(
                        (0, it_slot(0, 1, min(kt + PV_LAG, NKT - 1)) - 1, f))
            for pair in range(1, PAIRS):
                for tcix in range(NTC):         # K(p, t): scores at kt=4t
                    for f in qk_singles(1, pair, tcix):
                        pending.append((0, it_slot(pair, 0, 4 * tcix) - 2, f))
                for tcix in range(NTC):         # Q(p, t)
                    for f in qk_singles(0, pair, tcix):
                        pending.append((0, it_slot(pair, tcix) - 2, f))
            pending.sort(key=lambda x: x[1])

            cur = [0]

            def drain_auto(cap=8, base=1):
                # rate needed so every deadline in the near prefix is met
                need = 0
                for i, (rs, dl, _) in enumerate(pending):
                    if dl > cur[0] + 64:
                        break
                    slack = max(1, dl - cur[0])
                    need = max(need, -(-(i + 1) // slack))
                n = 0
                target = min(cap, max(base, need))
                while pending and n < target:
                    rs, dl, fn = pending[0]
                    if rs > cur[0]:
                        break
                    assert dl >= cur[0], f"missed deadline {dl} at {cur[0]}"
                    pending.pop(0)
                    fn()
                    n += 1

            # PV emission schedule: kt-slot -> list of lagged kt values
            pv_sched = [[] for _ in range(NKT)]
            for ktv in range(NKT):
                pv_sched[min(ktv + PV_LAG, NKT - 1)].append(ktv)

            def emit_pv(st, pso_h, half, kt):
                assert ("v", kt) in written
                hh = 2 * st["pair"] + half
                nc.tensor.matmul(
                    pso_h[0:DK + 1, :],
                    V[:, kt, hh, :],
                    st["phat"][:, kt, half * QC:(half + 1) * QC],
                    start=(kt == 0), stop=(kt == NKT - 1))

            def emit_tails(st):
                pso_h = st["pso"]
                for half in range(2):
                    l0 = small.tile([1, QC], F32, tag="l0", name="l0")
                    nc.vector.tensor_copy(l0, pso_h[half][DK:DK + 1, :])
                    r_sb = small.tile([1, QC], F32, tag="r", name="r_sb")
                    nc.vector.reciprocal_approx_fast(r_sb, l0)
                    r_bc = small.tile([DK, QC], F32, tag="rbc", name="r_bc")
                    nc.gpsimd.partition_broadcast(r_bc, r_sb)
                    nc.vector.tensor_mul(
                        o_t[st["qc"]][half * DK:(half + 1) * DK,
                                      st["pair"], :],
                        pso_h[half][0:DK, :], r_bc)
                tails_done[st["qc"]] += 1
                if st["pair"] == PAIRS - 1:
                    qc = st["qc"]
                    for oc in range(D // P):
                        for f in oproj_singles(qc, oc):
                            pending.append((cur[0] + 1, 1 << 30, f))

            # ---- main attention pipeline --------------------------------
            prev = None
            for pair in range(PAIRS):
                for qc in range(NQC):
                    qsl = slice(qc * QC, (qc + 1) * QC)
                    if apply_mask:
                        mt = opool.tile([P, NKT, QC], BF16, tag="mask")
                        nc.sync.dma_start(
                            mt, maskT.rearrange(
                                "(ko p) q -> p ko q", p=P)[:, :, qsl])
                    phat = phatp.tile([P, NKT, 2 * QC], BF16, tag="ph",
                                      name="phat")
                    self_st = {"qc": qc, "pair": pair, "phat": phat}
                    pso_h = None
                    for kt in range(NKT):
                        assert ("k", pair, kt * P // TC) in written
                        assert ("q", pair, qc) in written
                        ksl = slice(kt * P, (kt + 1) * P)
                        pss = ps.tile([P, 2 * QC], F32, tag="scores")
                        nc.tensor.matmul(
                            pss[:, 0:QC], KTt[0:DK, pair, ksl],
                            QT[0:DK, pair, qsl], start=True, stop=True)
                        nc.tensor.matmul(
                            pss[:, QC:2 * QC], KTt[DK:P, pair, ksl],
                            QT[DK:P, pair, qsl], start=True, stop=True)
                        if apply_mask:
                            nc.vector.tensor_add(
                                pss[:, 0:QC], pss[:, 0:QC], mt[:, kt])
                            nc.vector.tensor_add(
                                pss[:, QC:2 * QC], pss[:, QC:2 * QC],
                                mt[:, kt])
                        nc.scalar.activation(
                            phat[:, kt, :], pss,
                            mybir.ActivationFunctionType.Exp)
                        if prev is not None:
                            for ktv in pv_sched[kt]:
                                if ktv == 0:
                                    pso_h = [
                                        ps.tile([P, QC], F32, tag="pso",
                                                name=f"pso{h}")
                                        for h in range(2)]
                                emit_pv(prev, pso_h[0], 0, ktv)
                                emit_pv(prev, pso_h[1], 1, ktv)
                            if kt == NKT - 1:
                                prev["pso"] = pso_h
                                emit_tails(prev)
                        drain_auto()
                        cur[0] += 1
                    prev = {"qc": qc, "pair": pair, "phat": phat}

            # ---- epilogue ----------------------------------------------
            pso_h = [ps.tile([P, QC], F32, tag="pso", name=f"pso{h}")
                     for h in range(2)]
            for kt in range(NKT):
                emit_pv(prev, pso_h[0], 0, kt)
                emit_pv(prev, pso_h[1], 1, kt)
                drain_auto(cap=3, base=3)
                cur[0] += 1
            prev["pso"] = pso_h
            emit_tails(prev)
            while pending:
                rs, dl, fn = pending.pop(0)
                fn()

    nc.finalize()
    return nc


# --------------------------------------------------------------------------
# NTFF profiling shim (only used when kernel(..., _trace=True); provides
# antenv.axon_hooks so run_bass_kernel_spmd can capture profiles under axon).
def _install_ntff_shim():
    import contextlib, ctypes, sys, types
    try:
        import antenv.axon_hooks  # noqa: F401
        return
    except ImportError:
        pass
    so = "/opt/axon/libaxon_pjrt.so"
    try:
        lib = ctypes.CDLL(so)
    except OSError:
        return
    if not hasattr(lib, "axon_start_nrt_profile"):
        return
    lib.axon_start_nrt_profile.argtypes = [
        ctypes.POINTER(ctypes.c_int64), ctypes.c_size_t]
    lib.axon_start_nrt_profile.restype = ctypes.c_int64
    lib.axon_stop_nrt_profile.argtypes = [ctypes.c_char_p]
    lib.axon_stop_nrt_profile.restype = ctypes.c_int64

    @contextlib.contextmanager
    def _hook(output_dir, device_ids):
        import jax
        jax.devices()
        if device_ids:
            ids = (ctypes.c_int64 * len(device_ids))(*device_ids)
            rc = lib.axon_start_nrt_profile(ids, len(device_ids))
        else:
            rc = lib.axon_start_nrt_profile(None, 0)
        if rc != 0:
            raise RuntimeError(f"axon_start_nrt_profile rc={rc}")
        try:
            yield
        finally:
            n = lib.axon_stop_nrt_profile(str(output_dir).encode())
            print(f"ntff: {n} profile file(s) in {output_dir}", file=sys.stderr)

    import antenv
    mod = types.ModuleType("antenv.axon_hooks")
    mod.get_axon_ntff_profile_hook = lambda: _hook
    mod.set_axon_ntff_profile_hook = lambda h: None
    sys.modules["antenv.axon_hooks"] = mod
    antenv.axon_hooks = mod


def kernel(x, mask, Wq, bq, Wk, bk, Wv, bv, Wo, bo, _trace=False):
    global LAST_EXEC_NS
    x = np.ascontiguousarray(np.asarray(x, dtype=np.float32))
    mask = np.asarray(mask)
    Wq = np.asarray(Wq, dtype=np.float32)
    Wk = np.asarray(Wk, dtype=np.float32)
    Wv = np.asarray(Wv, dtype=np.float32)
    Wo = np.asarray(Wo, dtype=np.float32)
    bq = np.asarray(bq, dtype=np.float32)
    bk = np.asarray(bk, dtype=np.float32)
    bv = np.asarray(bv, dtype=np.float32)
    bo = np.asarray(bo, dtype=np.float32)

    scale = np.float32(1.0 / math.sqrt(DK))
    apply_mask = not bool((mask != 0).all())
    qkv_bias = bool(bq.any() or bk.any() or bv.any())

    nc = _build(apply_mask, qkv_bias)

    if apply_mask:
        mbias = np.where(mask == 0, np.float32(-30000.0), np.float32(0.0))
        # maskT[b][k, q] = mbias[b][q, k]
        maskT = np.ascontiguousarray(
            np.transpose(mbias, (0, 2, 1))).astype(ml_dtypes.bfloat16)

    in_maps = []
    for b in range(B):
        xT_np = np.ascontiguousarray(x[b].T).astype(ml_dtypes.bfloat16)  # [D, S]
        for g in range(HG):
            rows = slice(g * HD, (g + 1) * HD)
            wqkv_np = np.ascontiguousarray(np.concatenate(
                [Wq[rows].T * scale, Wk[rows].T, Wv[rows].T],
                axis=1)).astype(ml_dtypes.bfloat16)
            wo_np = np.ascontiguousarray(
                Wo[:, rows].T).astype(ml_dtypes.bfloat16)
            m = {"xT": xT_np, "wqkv": wqkv_np, "wo": wo_np}
            if apply_mask:
                m["maskT"] = maskT[b]
            if qkv_bias:
                m["qkb"] = np.ascontiguousarray(
                    np.stack([bq[rows] * scale, bk[rows]]))
                m["vb"] = np.ascontiguousarray(bv[rows])
            in_maps.append(m)

    if _trace:
        _install_ntff_shim()
    r = run_bass_kernel_spmd(nc, in_maps, list(range(NCORES)), trace=_trace)
    LAST_EXEC_NS = r.exec_time_ns

    y = np.empty((B, S, D), dtype=np.float32)
    for b in range(B):
        yT = (r.results[2 * b]["yT"].astype(np.float32)
              + r.results[2 * b + 1]["yT"].astype(np.float32))
        y[b] = yT.T + bo[None, :]
    return y
